# revision 25
# baseline (speedup 1.0000x reference)
"""Trainium2 Bass kernel for ExponentialConcordanceLoss.

Reference semantics (N = 8192):
    t = targets[:, 0]; e = targets[:, 1] != 0; s = preds
    mask[j, i] = (t[i] < t[j]) & e[i]            (all inputs finite)
    loss = sum_{j,i} mask * exp(s[j] - s[i]) / max(sum(mask), 1)

Factorization used on device:
    loss_sum = sum_j exp(s[j]) * (sum_i mask[j,i] * exp(-s[i]))
    count    = sum_{j,i} mask[j,i]

v3 layout: the i-axis keeps only event rows (non-events never fire the
mask), sorted by time; the j-axis is the full 8192 sorted by time.
Sorting is pure host-side layout prep - every compare/exp/product/
reduction still runs on device. For a 128-row i-block whose smallest
t' is v, every j with t_j <= v gives mask 0, so the block only needs
columns [jstart, 8192) where jstart = searchsorted(t_sorted, v) rounded
down to 128. Blocks are sorted by jstart and dealt round-robin into
"slots" of 8 (one block per core per slot), so the compiled program -
shared by all cores - has one static width per slot and the cores stay
perfectly balanced.

Per slot:
  pass1 (DVE, fp32 compare -> bf16 mask, 2x mode):
      m_T[i, j] = (t_j > t'_i) over [jstart, 8192), fused row-reduce
      gives exact pair counts
  pass2 (TensorEngine): psum[j, :] += m_T_chunk.T @ [w_hi, w_lo]
      (bf16 hi/lo split of exp(-s_i) keeps ~fp32 accuracy)
The t broadcast is split: DMA broadcast-reads the low half of the
sorted t row while GPSIMD partition-broadcasts the high half, tail
chunks first, so narrow (high-jstart) slots start almost immediately.
Epilogue: loss_rows = (hi+lo) * exp(s_j), reduce; the host sums the
8x[128,2] partials and divides.

The program is compiled per slot-width tuple (input-data metadata);
repeated calls with the same shape of data reuse the cache.
"""

import sys

if "/opt/trn_rl_repo" not in sys.path:
    sys.path.insert(0, "/opt/trn_rl_repo")

import numpy as np

N = 8192
NCORES = 8
NCH = N // 128         # j chunks of 128 (64)
CHUNKS = (0, 2048, 4096, 6144, 7168, 8192)  # broadcast chunk boundaries

_CACHE = {}


def _build(widths):
    """Trace the SPMD Bass program for the given per-slot widths
    (each a multiple of 128; slot q covers j in [N-width, N))."""
    import concourse.bass as bass
    import concourse.mybir as mybir

    f32 = mybir.dt.float32
    bf16 = mybir.dt.bfloat16
    Alu = mybir.AluOpType
    Act = mybir.ActivationFunctionType
    X = mybir.AxisListType.X

    nslots = len(widths)
    jstarts = [N - w for w in widths]
    # pieces: (slot, chunk, lo, hi), ordered tail-chunk-first then by slot,
    # so work starts as soon as each broadcast chunk lands
    pieces = []
    for ci in range(len(CHUNKS) - 2, -1, -1):
        for q in range(nslots):
            lo = max(jstarts[q], CHUNKS[ci])
            hi = CHUNKS[ci + 1]
            if lo < hi:
                pieces.append((q, ci, lo, hi))
    npieces = len(pieces)

    nc = bass.Bass()

    tflat_d = nc.dram_tensor("tflat", [N], f32, kind="ExternalInput")
    ploc_d = nc.dram_tensor("ploc", [128, 3 * nslots], f32, kind="ExternalInput")
    sjb_d = nc.dram_tensor("sjb", [128, NCH], f32, kind="ExternalInput")
    out_d = nc.dram_tensor("out", [128, 2], f32, kind="ExternalOutput")

    from contextlib import ExitStack

    with ExitStack() as ctx:
        en = ctx.enter_context
        ploc_s = en(nc.sbuf_tensor([128, 3 * nslots], f32))
        sjb_s = en(nc.sbuf_tensor([128, NCH], f32))
        tmp8 = en(nc.sbuf_tensor([128, nslots], f32))
        texc_loc = en(nc.sbuf_tensor([128, nslots], f32))
        w_f32 = en(nc.sbuf_tensor([128, nslots], f32))
        whi = en(nc.sbuf_tensor([128, nslots], bf16))
        wlo_f = en(nc.sbuf_tensor([128, nslots], f32))
        wpair = en(nc.sbuf_tensor([128, 2 * nslots], bf16))
        vjb = en(nc.sbuf_tensor([128, NCH], f32))
        cntT = en(nc.sbuf_tensor([128, npieces], f32))
        lrows = en(nc.sbuf_tensor([128, NCH], f32))
        red = en(nc.sbuf_tensor([128, 2], f32))
        pef_s = en(nc.sbuf_tensor([128, 2 * NCH], f32))
        tjb = en(nc.sbuf_tensor([128, N], f32))
        mA = en(nc.sbuf_tensor([128, N], bf16))
        mB = en(nc.sbuf_tensor([128, N], bf16))
        ptile = en(nc.psum_tensor([128, 2 * NCH], f32))
        dsem = en(nc.semaphore())    # ploc load
        sjsem = en(nc.semaphore())   # sjb load
        csems = [en(nc.semaphore(f"csem{i}")) for i in range(len(CHUNKS) - 1)]  # broadcast chunks
        outsem = en(nc.semaphore())
        asem = en(nc.semaphore())
        vv = en(nc.semaphore())
        pesem = en(nc.semaphore())
        block = en(nc.Block())
        mbufs = [mA, mB]

        VV_WPAIR = 7
        VV_P1 = lambda p: VV_WPAIR + p + 1   # pass1 piece p done
        VV_DONE = VV_WPAIR + npieces + 5

        @block.sync
        def _(sync):
            # ploc first (unblocks ACT exp + DVE setup), then the small
            # tail chunk of the t broadcast (unblocks the narrow slots),
            # then the rest, tail first; one sem per chunk keeps
            # increments deterministic without chaining
            nch = len(CHUNKS) - 1
            sync.dma_start(ploc_s[:], ploc_d[:]).then_inc(dsem, 16)
            sync.dma_start(
                tjb[:, CHUNKS[nch - 1] : CHUNKS[nch]],
                tflat_d[None, CHUNKS[nch - 1] : CHUNKS[nch]].partition_broadcast(128),
            ).then_inc(csems[nch - 1], 16)
            sync.dma_start(sjb_s[:], sjb_d[:]).then_inc(sjsem, 16)
            for ci in range(nch - 2, -1, -1):
                sync.dma_start(
                    tjb[:, CHUNKS[ci] : CHUNKS[ci + 1]],
                    tflat_d[None, CHUNKS[ci] : CHUNKS[ci + 1]].partition_broadcast(128),
                ).then_inc(csems[ci], 16)
            sync.wait_ge(vv, VV_DONE)
            sync.dma_start(out_d[:], red[:, 0:2]).then_inc(outsem, 16)
            sync.wait_ge(outsem, 16)

        @block.scalar
        def _(scalar):
            scalar.wait_ge(dsem, 16)
            scalar.activation(w_f32[:], ploc_s[:, 2 * nslots : 3 * nslots], Act.Exp, scale=-1.0).then_inc(
                asem, 1
            )
            scalar.wait_ge(sjsem, 16)
            scalar.activation(vjb[:], sjb_s[:], Act.Exp).then_inc(asem, 1)

        @block.vector
        def _(vector):
            n = 0

            def step(ins):
                nonlocal n
                n += 1
                ins.then_inc(vv, 1)

            vector.wait_ge(dsem, 16)
            # t'_i = t_i + 1e30 * (e_i == 0)
            step(vector.tensor_scalar(
                out=tmp8[:], in0=ploc_s[:, nslots : 2 * nslots], scalar1=0.0,
                scalar2=1e30, op0=Alu.is_equal, op1=Alu.mult,
            ))
            vector.wait_ge(vv, n)
            step(vector.tensor_add(texc_loc[:], tmp8[:], ploc_s[:, 0:nslots]))
            # bf16 hi/lo split of w = exp(-s_i)
            vector.wait_ge(asem, 1)
            step(vector.tensor_copy(whi[:], w_f32[:]))
            vector.wait_ge(vv, n)
            step(vector.tensor_sub(wlo_f[:], w_f32[:], whi[:]))
            vector.wait_ge(vv, n)
            step(vector.tensor_copy(wpair[:, 0 : 2 * nslots : 2], whi[:]))
            vector.wait_ge(vv, n)
            step(vector.tensor_copy(wpair[:, 1 : 2 * nslots : 2], wlo_f[:]))
            vector.wait_ge(vv, n)
            step(vector.memset(ptile[:], 0.0))
            assert n == VV_WPAIR
            for p, (q, ci, lo, hi) in enumerate(pieces):
                vector.wait_ge(csems[ci], 16)
                if p >= 2:
                    vector.wait_ge(pesem, p - 1)  # PE done with this region
                vector.wait_ge(vv, n)
                step(vector.tensor_scalar(
                    out=mbufs[q % 2][:, lo:hi], in0=tjb[:, lo:hi],
                    scalar1=texc_loc[:, q : q + 1], scalar2=None,
                    op0=Alu.is_gt, op1=Alu.add,
                    accum_out=cntT[:, p : p + 1],
                ))
                assert n == VV_P1(p)
            # epilogue (only one PSUM operand allowed per DVE op)
            vector.wait_ge(pesem, npieces)
            step(vector.tensor_copy(pef_s[:], ptile[:]))
            vector.wait_ge(vv, n)
            step(vector.tensor_add(
                lrows[:], pef_s[:, 0 : 2 * NCH : 2], pef_s[:, 1 : 2 * NCH : 2]
            ))
            vector.wait_ge(asem, 2)
            vector.wait_ge(vv, n)
            step(vector.tensor_mul(lrows[:], lrows[:], vjb[:]))
            vector.wait_ge(vv, n)
            step(vector.reduce_sum(out=red[:, 0:1], in_=lrows[:], axis=X))
            vector.wait_ge(vv, n)
            step(vector.reduce_sum(out=red[:, 1:2], in_=cntT[:], axis=X))
            assert n == VV_DONE

        @block.tensor
        def _(tensor):
            tensor.wait_ge(vv, VV_WPAIR)
            first = True
            for p, (q, ci, lo, hi) in enumerate(pieces):
                tensor.wait_ge(vv, VV_P1(p))
                m = mbufs[q % 2]
                for c in range(lo // 128, hi // 128):
                    # 'start' marks the whole 2KB psum zero-region as
                    # pending-zero, so issue it exactly once; each column's
                    # first touch then auto-zeroes (memset covers columns no
                    # matmul ever writes).
                    ins = tensor.matmul(
                        ptile[:, 2 * c : 2 * c + 2],
                        m[:, 128 * c : 128 * (c + 1)],
                        wpair[:, 2 * q : 2 * q + 2],
                        start=first,
                        stop=(p == npieces - 1 and c == hi // 128 - 1),
                        skip_group_check=True,
                    )
                    first = False
                ins.then_inc(pesem, 1)

    return nc


def _plan(preds, targets):
    """Host-side layout prep: sort, block, and slot the work."""
    t = np.ascontiguousarray(targets[:, 0], dtype=np.float32)
    e = np.ascontiguousarray(targets[:, 1], dtype=np.float32)
    s = np.ascontiguousarray(preds, dtype=np.float32).reshape(-1)

    orderj = np.argsort(t, kind="stable")
    t_j = t[orderj]
    s_j = s[orderj]

    ev = np.flatnonzero(e != 0.0)
    if len(ev) == 0:
        return None
    ev = ev[np.argsort(t[ev], kind="stable")]
    nblocks = -(-len(ev) // 128)
    nblocks_pad = -(-nblocks // NCORES) * NCORES

    # per-block (t, e, s) rows and jstart
    bt = np.zeros((nblocks_pad, 128), np.float32)
    be = np.zeros((nblocks_pad, 128), np.float32)
    bs = np.zeros((nblocks_pad, 128), np.float32)
    jstart = np.full(nblocks_pad, N, np.int64)
    for b in range(nblocks):
        idx = ev[b * 128 : (b + 1) * 128]
        k = len(idx)
        bt[b, :k] = t[idx]
        be[b, :k] = 1.0
        bs[b, :k] = s[idx]
        js = int(np.searchsorted(t_j, t[idx[0]], side="right"))
        jstart[b] = (js // 128) * 128

    # deal blocks (sorted by jstart desc) into slots of NCORES
    order_b = np.argsort(-jstart, kind="stable")
    nslots = nblocks_pad // NCORES
    widths = []
    slot_blocks = []
    for q in range(nslots):
        grp = order_b[q * NCORES : (q + 1) * NCORES]
        js = int(jstart[grp].min())
        w = max(128, N - js)
        widths.append(w)
        slot_blocks.append(grp)

    maps = []
    shared = {
        "tflat": t_j,
        "sjb": np.ascontiguousarray(s_j.reshape(NCH, 128).T),
    }
    for c in range(NCORES):
        ploc = np.zeros((128, 3 * nslots), np.float32)
        for q in range(nslots):
            b = slot_blocks[q][c]
            ploc[:, q] = bt[b]
            ploc[:, nslots + q] = be[b]
            ploc[:, 2 * nslots + q] = bs[b]
        maps.append(dict(shared, ploc=ploc))
    return tuple(widths), maps


def _combine(results):
    loss_sum = 0.0
    count = 0.0
    for r in results:
        part = np.asarray(r["out"], dtype=np.float64)
        loss_sum += part[:, 0].sum()
        count += part[:, 1].sum()
    return np.array(np.float32(loss_sum) / np.float32(max(count, 1.0)),
                    dtype=np.float32)


def kernel(preds, targets):
    from concourse.bass_utils import run_bass_kernel_spmd

    plan = _plan(preds, targets)
    if plan is None:
        return np.array(0.0, dtype=np.float32)
    widths, maps = plan
    if widths not in _CACHE:
        _CACHE[widths] = _build(widths)
    nc = _CACHE[widths]
    res = run_bass_kernel_spmd(nc, maps, list(range(NCORES)))
    return _combine(res.results)
